# revision 14
# baseline (speedup 1.0000x reference)
"""MoE (top-2 of 8 experts) Trainium2 kernel.

Strategy: expert-parallel across 8 NeuronCores with token rebalancing.
The router (8192x1024 @ 1024x8 + top-k) is tiny, so it runs on host in
float64 (verified to reproduce the fp32 reference ranking). Each core
processes 2176 token slots = 16 "primary" blocks of 128 for its own
expert plus 1 "guest" block carrying another expert's overflow (the
per-expert token counts for the benchmark routing are 1868..2252, so a
uniform 18-block capacity wastes ~6% PE time; 17 blocks with guest
rebalancing is the minimum uniform SPMD capacity). The FFN runs in bf16
(full PE rate, fast weight load, half the DMA bytes of fp32; ~4e-3 rel
err vs the 2e-2 gate) with exact-erf Gelu on ScalarE; the host
scatter-adds the two expert contributions per token.

Device layout: stage 1 computes h.T = gelu(W1.T @ x.T + b1) with W1
blocks stationary; stage 2 uses h.T 128x128 blocks as the stationary
operand streaming two 512-wide W2 chunks per load (halving weight-load
count) and produces y directly in [token, E] layout. The 1/k scale is
folded into W2 on host (exact for k=2); b2/k is added with a DVE
tensor_add from a partition-replicated tile.

DMA queues: token/weight loads go out on the sync engine's HWDGE ring;
y stores ride the otherwise-idle gpsimd (SWDGE) queue, so input
prefetch is never queued behind store triggers that wait on compute.
The guest expert's weights are DMA'd behind the primary set and only
consumed at the end of the pass, hiding their load under primary
compute. PSUM: 4 banks for stage-1 accumulation chains, 4 for stage-2
so consecutive 128-token blocks alternate bank pairs and the PE never
waits on the DVE drain.
"""

import sys

sys.path.insert(0, "/opt/trn_rl_repo")

import math

import numpy as np

_B, _SEQ, _E, _H, _NE = 4, 2048, 1024, 1536, 8
_T = _B * _SEQ
_CAPP = 2048  # primary region (16 blocks of 128, own expert)
_CAPG = 128   # guest region (1 block, possibly another expert's overflow)
_CAP = _CAPP + _CAPG
_NCORES = 8
_P = 128

_nc_cache: dict = {}
_LOOP_BODY_REPS = [1]  # timing-only knob for the For_i variant


def _build_nc(
    inv_k: float,
    repeat: int = 1,
    loop_n: int = 0,
    ps1_bufs: int = 4,
    ps2_bufs: int = 4,
    body: str = "full",  # loop-mode diagnostics: full|dma|compute|compute_nodma
):
    """repeat>1 repeats the compute (timing); repeat=0 builds an I/O-identical
    near-no-op NEFF used as the timing baseline; loop_n>0 wraps the pass in a
    device-side For_i loop (timing only)."""
    from contextlib import ExitStack

    import concourse.tile as tile
    from concourse import bacc, mybir

    f32 = mybir.dt.float32
    bf = mybir.dt.bfloat16
    KO1 = _E // _P   # 8  k-tiles for layer-1 contraction
    HT = _H // _P    # 12 h-tiles (layer-1 out / layer-2 contraction)
    EC = _E // 512   # 2  512-wide E chunks in stage 2
    colt = [(0, 512), (512, 512), (1024, 512), (1536, 512)]  # primary tiles
    gcol = (_CAPP, _CAPG)                                    # guest tile

    nc = bacc.Bacc("TRN2", target_bir_lowering=False, debug=False)
    xt_d = nc.dram_tensor("xt", [_E, _CAP], bf, kind="ExternalInput").ap()
    # a = primary expert weights, b = guest expert weights
    # w2 is host-scaled by 1/k (exact for k a power of two); b1 is laid out
    # [128, HT] (bias hi per column), b2 replicated across partitions.
    wd = {}
    for s in ("a", "b"):
        wd[s] = dict(
            w1=nc.dram_tensor(f"w1{s}", [_E, _H], bf, kind="ExternalInput").ap(),
            w2=nc.dram_tensor(f"w2{s}", [_H, _E], bf, kind="ExternalInput").ap(),
            b1=nc.dram_tensor(f"b1{s}", [_P, HT], f32, kind="ExternalInput").ap(),
            b2=nc.dram_tensor(f"b2{s}", [_P, _E], f32, kind="ExternalInput").ap(),
        )
    # y in natural [token, E] layout
    y_d = nc.dram_tensor("y", [_CAP, _E], bf, kind="ExternalOutput").ap()

    with tile.TileContext(nc) as tc:
        with ExitStack() as ctx:
            wpool = ctx.enter_context(tc.tile_pool(name="w", bufs=1))
            cpool = ctx.enter_context(tc.tile_pool(name="c", bufs=1))
            xpool = ctx.enter_context(tc.tile_pool(name="x", bufs=5))
            hpool = ctx.enter_context(tc.tile_pool(name="h", bufs=2))
            ypool = ctx.enter_context(tc.tile_pool(name="y", bufs=4))
            ps1 = ctx.enter_context(tc.tile_pool(name="ps1", bufs=ps1_bufs, space="PSUM"))
            ps2 = ctx.enter_context(tc.tile_pool(name="ps2", bufs=ps2_bufs, space="PSUM"))

            if repeat == 0 and not loop_n:
                t = cpool.tile([_P, 4], f32, tag="nop")
                nc.sync.dma_start(t[:], wd["a"]["b1"][:, 0:4])
                o = cpool.tile([_P, 4], f32, tag="nop_o")
                nc.vector.tensor_copy(o[:], t[:])
                nc.gpsimd.dma_start(y_d[0:_P, 0:4], o[:])
            else:
                ws = {}
                for s in ("a", "b"):
                    ws[s] = dict(
                        w1=[wpool.tile([_P, _H], bf, tag=f"w1{s}_{ko}",
                                       name=f"w1{s}_{ko}") for ko in range(KO1)],
                        w2=[wpool.tile([_P, _E], bf, tag=f"w2{s}_{hi}",
                                       name=f"w2{s}_{hi}") for hi in range(HT)],
                        b1=cpool.tile([_P, HT], f32, tag=f"b1{s}", name=f"b1{s}_sb"),
                        b2=cpool.tile([_P, _E], f32, tag=f"b2{s}", name=f"b2{s}_sb"),
                    )

                def dma_w(s, parts=("w1", "b1", "w2", "b2")):
                    for p in parts:
                        if p == "w1":
                            for ko in range(KO1):
                                nc.sync.dma_start(
                                    ws[s]["w1"][ko][:],
                                    wd[s]["w1"][ko * _P : (ko + 1) * _P, :],
                                )
                        elif p == "w2":
                            for hi in range(HT):
                                nc.sync.dma_start(
                                    ws[s]["w2"][hi][:],
                                    wd[s]["w2"][hi * _P : (hi + 1) * _P, :],
                                )
                        else:
                            nc.sync.dma_start(ws[s][p][:], wd[s][p][:, :])

                def dma_xt(c0, nt, tag2, interleave=None, ilv_first=False):
                    tiles = []
                    for ko in range(KO1):
                        if interleave is not None and ilv_first:
                            interleave(ko)
                        tt = xpool.tile([_P, nt], bf, tag=f"xt{ko}_{nt}",
                                        name=f"xt_{tag2}_{ko}")
                        nc.sync.dma_start(
                            tt[:], xt_d[ko * _P : (ko + 1) * _P, c0 : c0 + nt]
                        )
                        if interleave is not None and not ilv_first:
                            interleave(ko)
                        tiles.append(tt)
                    return lambda ko: tiles[ko][:]

                def alloc_h(nt, tag2):
                    tiles = [
                        hpool.tile([_P, nt], bf, tag=f"h{hi}_{nt}",
                                   name=f"h_{tag2}_{hi}")
                        for hi in range(HT)
                    ]
                    return (lambda hi: tiles[hi][:],
                            lambda hk, cb: tiles[hk][:, cb * _P : (cb + 1) * _P])

                def stage1_koouter(xt_sb, nt, s):
                    # ko-outer half-passes: PE starts as soon as w1 block 0 lands
                    h_w, h_r = alloc_h(nt, "t0")
                    for half in range(2):
                        accs = [
                            ps1.tile([_P, nt], f32, tag="ps1", name=f"ps_h{half}_{i}")
                            for i in range(6)
                        ]
                        for ko in range(KO1):
                            for i in range(6):
                                hi = half * 6 + i
                                nc.tensor.matmul(
                                    accs[i][:],
                                    ws[s]["w1"][ko][:, hi * _P : (hi + 1) * _P],
                                    xt_sb(ko),
                                    start=(ko == 0),
                                    stop=(ko == KO1 - 1),
                                )
                        for i in range(6):
                            hi = half * 6 + i
                            nc.scalar.activation(
                                h_w(hi),
                                accs[i][:],
                                mybir.ActivationFunctionType.Gelu,
                                bias=ws[s]["b1"][:, hi : hi + 1],
                                scale=1.0,
                            )
                    return h_w, h_r

                def stage1(xt_sb, nt, tag2, s):
                    h_w, h_r = alloc_h(nt, tag2)
                    for hi in range(HT):
                        acc = ps1.tile([_P, nt], f32, tag="ps1", name=f"p1_{tag2}_{hi}")
                        for ko in range(KO1):
                            nc.tensor.matmul(
                                acc[:],
                                ws[s]["w1"][ko][:, hi * _P : (hi + 1) * _P],
                                xt_sb(ko),
                                start=(ko == 0),
                                stop=(ko == KO1 - 1),
                            )
                        nc.scalar.activation(
                            h_w(hi),
                            acc[:],
                            mybir.ActivationFunctionType.Gelu,
                            bias=ws[s]["b1"][:, hi : hi + 1],
                            scale=1.0,
                        )
                    return h_w, h_r

                def stage2(h_r, c0, nt, tag2, s, dma_out=True):
                    # stationary = h.T 128x128 block (one load, 2 MMs of N=512)
                    for cb in range(nt // _P):
                        accs = [
                            ps2.tile([_P, 512], f32, tag="ps2",
                                     name=f"p2_{tag2}_{cb}_{ec}")
                            for ec in range(EC)
                        ]
                        for hk in range(HT):
                            lhs = h_r(hk, cb)
                            for ec in range(EC):
                                nc.tensor.matmul(
                                    accs[ec][:],
                                    lhs,
                                    ws[s]["w2"][hk][:, ec * 512 : (ec + 1) * 512],
                                    start=(hk == 0),
                                    stop=(hk == HT - 1),
                                )
                        for ec in range(EC):
                            y_sb = ypool.tile([_P, 512], bf, tag="y",
                                              name=f"y_{tag2}_{cb}_{ec}")
                            nc.vector.tensor_add(
                                y_sb[:],
                                accs[ec][:],
                                ws[s]["b2"][:, ec * 512 : (ec + 1) * 512],
                            )
                            if dma_out:
                                # HWDGE via the ACT ring: a gpsimd (SWDGE)
                                # store here deadlocks against the DVE adds
                                # for the shared SBUF port pair (descriptor
                                # starvation), stalling the drain.
                                nc.scalar.dma_start(
                                    y_d[
                                        c0 + cb * _P : c0 + (cb + 1) * _P,
                                        ec * 512 : (ec + 1) * 512,
                                    ],
                                    y_sb[:],
                                )

                def emit_pass(tag):
                    xts = [dma_xt(c0, nt, f"{tag}_{c0}") for c0, nt in colt]
                    xtg = dma_xt(gcol[0], gcol[1], f"{tag}_g")
                    for t, (c0, nt) in enumerate(colt):
                        _, h_r = stage1(xts[t], nt, f"{tag}_{c0}", "a")
                        stage2(h_r, c0, nt, f"{tag}_{c0}", "a")
                    _, h_r = stage1(xtg, gcol[1], f"{tag}_g", "b")
                    stage2(h_r, gcol[0], gcol[1], f"{tag}_g", "b")

                if loop_n:
                    # weights resident; body re-loads xt and writes y
                    dma_w("a")
                    dma_w("b")
                    if body in ("compute", "compute_nodma", "s1only", "pemm_deep",
                                "pemm_alt", "pemm_single"):
                        xts_res = [dma_xt(c0, nt, f"res{c0}") for c0, nt in colt]
                        xtg_res = dma_xt(gcol[0], gcol[1], "resg")
                    if body in ("s2only", "s2only_nodve"):
                        xts_res = [dma_xt(colt[0][0], colt[0][1], "res0")]
                        h_res = stage1(xts_res[0], 512, "res0", "a")[1]
                    body_reps = int(_LOOP_BODY_REPS[0])
                    with tc.For_i(0, loop_n, 1) as _i:
                        for br in range(body_reps):
                            if body == "full":
                                emit_pass(f"L{br}")
                            elif body == "dma":
                                for c0, nt in colt:
                                    dma_xt(c0, nt, f"L{br}_{c0}")
                                dma_xt(gcol[0], gcol[1], f"L{br}_g")
                            elif body in ("compute", "compute_nodma"):
                                do = body == "compute"
                                for t, (c0, nt) in enumerate(colt):
                                    _, h_r = stage1(
                                        xts_res[t], nt, f"L{br}_{c0}", "a")
                                    stage2(h_r, c0, nt, f"L{br}_{c0}", "a",
                                           dma_out=do)
                                _, h_r = stage1(xtg_res, gcol[1], f"L{br}_g", "b")
                                stage2(h_r, gcol[0], gcol[1], f"L{br}_g", "b",
                                       dma_out=do)
                            elif body == "s1only":
                                # stage-1 shape only (PE chains + ACT gelu)
                                for t, (c0, nt) in enumerate(colt):
                                    stage1(xts_res[t], nt, f"L{br}_{c0}", "a")
                                stage1(xtg_res, gcol[1], f"L{br}_g", "b")
                            elif body == "s2only":
                                # stage-2 shape + DVE adds, one resident h tile
                                for t in range(4):
                                    stage2(h_res, t * 512, 512, f"L{br}_{t}", "a",
                                           dma_out=False)
                            elif body == "s2only_nodve":
                                for t in range(4):
                                    for cb in range(4):
                                        accs = [
                                            ps2.tile([_P, 512], f32, tag="ps2",
                                                     name=f"pp_{br}_{t}_{cb}_{ec}")
                                            for ec in range(EC)
                                        ]
                                        for hk in range(HT):
                                            lhs = h_res(hk, cb)
                                            for ec in range(EC):
                                                nc.tensor.matmul(
                                                    accs[ec][:], lhs,
                                                    ws["a"]["w2"][hk][:, ec * 512 : (ec + 1) * 512],
                                                    start=(hk == 0),
                                                    stop=(hk == HT - 1),
                                                )
                            elif body in ("pemm_deep", "pemm_alt", "pemm_single"):
                                # pure PE chains, no consumers at all
                                xb = xts_res[0]
                                for g in range(17):
                                    if body == "pemm_single":
                                        # one 24-deep chain into a single bank
                                        acc = ps2.tile(
                                            [_P, 512], f32, tag="ps2",
                                            name=f"pm_{br}_{g}")
                                        for j in range(24):
                                            hk = j % HT
                                            nc.tensor.matmul(
                                                acc[:], xb(0)[:, 0:_P],
                                                ws["a"]["w2"][hk][:, 0:512],
                                                start=(j == 0),
                                                stop=(j == 23),
                                            )
                                        continue
                                    accs = [
                                        ps2.tile([_P, 512], f32, tag="ps2",
                                                 name=f"pm_{br}_{g}_{ec}")
                                        for ec in range(EC)
                                    ]
                                    if body == "pemm_deep":
                                        for ec in range(EC):
                                            for hk in range(HT):
                                                nc.tensor.matmul(
                                                    accs[ec][:], xb(0)[:, 0:_P],
                                                    ws["a"]["w2"][hk][:, ec * 512 : (ec + 1) * 512],
                                                    start=(hk == 0),
                                                    stop=(hk == HT - 1),
                                                )
                                    else:
                                        for hk in range(HT):
                                            for ec in range(EC):
                                                nc.tensor.matmul(
                                                    accs[ec][:], xb(0)[:, 0:_P],
                                                    ws["a"]["w2"][hk][:, ec * 512 : (ec + 1) * 512],
                                                    start=(hk == 0),
                                                    stop=(hk == HT - 1),
                                                )
                else:
                    for r in range(repeat):
                        if r == 0:
                            # prelude issue order: w1a/xt0 interleaved so the PE
                            # can start immediately; b1a; then xt1..3 with w2a
                            # trickled in between; b2a; guest xt; the guest
                            # weight set last (consumed at the end of the pass).
                            xt0 = dma_xt(
                                colt[0][0], colt[0][1], "r0t0",
                                interleave=lambda ko: nc.sync.dma_start(
                                    ws["a"]["w1"][ko][:],
                                    wd["a"]["w1"][ko * _P : (ko + 1) * _P, :],
                                ),
                                ilv_first=True,
                            )
                            nc.sync.dma_start(ws["a"]["b1"][:], wd["a"]["b1"][:, :])
                            xts = [xt0]
                            w2i = iter(range(HT))

                            def ilv(ko):
                                hi = next(w2i, None)
                                if hi is not None:
                                    nc.sync.dma_start(
                                        ws["a"]["w2"][hi][:],
                                        wd["a"]["w2"][hi * _P : (hi + 1) * _P, :],
                                    )

                            for c0, nt in colt[1:]:
                                xts.append(dma_xt(c0, nt, f"r0c{c0}", interleave=ilv))
                            for hi in w2i:
                                nc.sync.dma_start(
                                    ws["a"]["w2"][hi][:],
                                    wd["a"]["w2"][hi * _P : (hi + 1) * _P, :],
                                )
                            nc.sync.dma_start(ws["a"]["b2"][:], wd["a"]["b2"][:, :])
                            xtg = dma_xt(gcol[0], gcol[1], "r0g")
                            dma_w("b")
                            _, h0r = stage1_koouter(xts[0], colt[0][1], "a")
                            _, h1r = stage1(xts[1], colt[1][1], "r0t1", "a")
                            stage2(h0r, colt[0][0], colt[0][1], "r0t0", "a")
                            stage2(h1r, colt[1][0], colt[1][1], "r0t1", "a")
                            for t, (c0, nt) in enumerate(colt[2:], start=2):
                                _, h_r = stage1(xts[t], nt, f"r0c{c0}", "a")
                                stage2(h_r, c0, nt, f"r0c{c0}", "a")
                            _, h_r = stage1(xtg, gcol[1], "r0g", "b")
                            stage2(h_r, gcol[0], gcol[1], "r0g", "b")
                        else:
                            emit_pass(f"r{r}")
    nc.compile()
    return nc


def _make_runner(nc, n_cores):
    """Persistent-jit SPMD runner (modeled on bass2jax.run_bass_via_pjrt)."""
    import jax
    import numpy as _np
    from jax.sharding import Mesh, PartitionSpec
    from jax.experimental.shard_map import shard_map

    from concourse import mybir
    from concourse.bass2jax import (
        _bass_exec_p,
        install_neuronx_cc_hook,
        partition_id_tensor,
    )

    install_neuronx_cc_hook()

    partition_name = nc.partition_id_tensor.name if nc.partition_id_tensor else None
    in_names: list = []
    out_names: list = []
    out_avals: list = []
    zero_outs: list = []
    for alloc in nc.m.functions[0].allocations:
        if not isinstance(alloc, mybir.MemoryLocationSet):
            continue
        name = alloc.memorylocations[0].name
        if alloc.kind == "ExternalInput":
            if name != partition_name:
                in_names.append(name)
        elif alloc.kind == "ExternalOutput":
            shape = tuple(alloc.tensor_shape)
            dtype = mybir.dt.np(alloc.dtype)
            out_names.append(name)
            out_avals.append(jax.core.ShapedArray(shape, dtype))
            zero_outs.append(_np.zeros(shape, dtype))
    n_params = len(in_names)
    n_outs = len(out_avals)
    all_in_names = in_names + out_names
    if partition_name is not None:
        all_in_names = all_in_names + [partition_name]

    def _body(*args):
        operands = list(args)
        if partition_name is not None:
            operands.append(partition_id_tensor())
        outs = _bass_exec_p.bind(
            *operands,
            out_avals=tuple(out_avals),
            in_names=tuple(all_in_names),
            out_names=tuple(out_names),
            lowering_input_output_aliases=(),
            sim_require_finite=True,
            sim_require_nnan=True,
            nc=nc,
        )
        return tuple(outs)

    devices = jax.devices()[:n_cores]
    assert len(devices) == n_cores
    mesh = Mesh(_np.asarray(devices), ("core",))
    in_specs = (PartitionSpec("core"),) * (n_params + n_outs)
    out_specs = (PartitionSpec("core"),) * n_outs
    donate = tuple(range(n_params, n_params + n_outs))
    sharded = jax.jit(
        shard_map(
            _body, mesh=mesh, in_specs=in_specs, out_specs=out_specs, check_rep=False
        ),
        donate_argnums=donate,
        keep_unused=True,
    )

    def run(in_maps):
        concat_in = [
            _np.concatenate([_np.asarray(in_maps[c][nm]) for c in range(n_cores)], axis=0)
            for nm in in_names
        ]
        concat_zeros = [
            _np.zeros((n_cores * z.shape[0], *z.shape[1:]), z.dtype) for z in zero_outs
        ]
        out_arrs = sharded(*concat_in, *concat_zeros)
        out_arrs = [_np.asarray(o) for o in out_arrs]
        return [
            {
                nm: out_arrs[i].reshape(n_cores, *out_avals[i].shape)[c]
                for i, nm in enumerate(out_names)
            }
            for c in range(n_cores)
        ]

    return run


def _route(flat, Wr, br, k):
    logits = flat.astype(np.float64) @ Wr.astype(np.float64) + br.astype(np.float64)
    order = np.argsort(-logits, axis=1, kind="stable")
    return order[:, :k]


def _host_expert(xe, W1e, b1e, W2e, b2e):
    h = xe.astype(np.float64) @ W1e.astype(np.float64) + b1e.astype(np.float64)
    try:
        from scipy.special import erf
    except ImportError:
        erf = np.vectorize(math.erf)
    h = 0.5 * h * (1.0 + erf(h / math.sqrt(2.0)))
    return h @ W2e.astype(np.float64) + b2e.astype(np.float64)


def _have_axon_devices():
    try:
        import jax

        return (
            sum(d.platform in ("axon", "neuron") for d in jax.devices()) >= _NCORES
        )
    except Exception:
        return False


def _pack(idx_lists):
    """Assign tokens to cores: expert c's first 2048 tokens are core c's
    primary region; overflow is split into <=128-token chunks, one per
    guest slot, round-robin over cores. Returns per-core primary/guest
    index arrays plus any chunks that didn't fit (host fallback)."""
    prim = []
    chunks = []
    for e in range(_NE):
        idx = idx_lists[e]
        prim.append(idx[:_CAPP])
        rest = idx[_CAPP:]
        for i in range(0, len(rest), _CAPG):
            chunks.append((e, rest[i : i + _CAPG]))
    guest = [None] * _NCORES
    unplaced = []
    free = list(range(_NCORES))
    for ch in chunks:
        if free:
            guest[free.pop(0)] = ch
        else:
            unplaced.append(ch)
    return prim, guest, unplaced


def _prepare(inputs):
    import ml_dtypes

    bf16 = ml_dtypes.bfloat16
    x = np.asarray(inputs["x"], np.float32)
    Wr = np.asarray(inputs["Wr"], np.float32)
    br = np.asarray(inputs["br"], np.float32)
    W1 = np.asarray(inputs["W1"], np.float32)
    b1 = np.asarray(inputs["b1"], np.float32)
    W2 = np.asarray(inputs["W2"], np.float32)
    b2 = np.asarray(inputs["b2"], np.float32)
    k = int(np.asarray(inputs["k"]))
    assert x.shape == (_B, _SEQ, _E), x.shape

    flat = x.reshape(_T, _E)
    topk = _route(flat, Wr, br, k)
    flatT = np.ascontiguousarray(flat.T)

    idx_lists = [np.nonzero((topk == e).any(axis=1))[0] for e in range(_NE)]
    prim, guest, unplaced = _pack(idx_lists)

    w1b = [W1[e].astype(bf16) for e in range(_NE)]
    w2b = [(W2[e] / k).astype(bf16) for e in range(_NE)]
    b1p = [np.ascontiguousarray(b1[e].reshape(_H // _P, _P).T) for e in range(_NE)]
    b2r = [np.broadcast_to(b2[e] / k, (_P, _E)).copy() for e in range(_NE)]

    in_maps = []
    for c in range(_NE):
        xt = np.zeros((_E, _CAP), bf16)
        xt[:, : len(prim[c])] = flatT[:, prim[c]]
        ge = c
        if guest[c] is not None:
            ge, gidx = guest[c]
            xt[:, _CAPP : _CAPP + len(gidx)] = flatT[:, gidx]
        in_maps.append(
            {
                "xt": xt,
                "w1a": w1b[c], "w2a": w2b[c], "b1a": b1p[c], "b2a": b2r[c],
                "w1b": w1b[ge], "w2b": w2b[ge], "b1b": b1p[ge], "b2b": b2r[ge],
            }
        )
    return flat, k, in_maps, prim, guest, unplaced, (W1, b1, W2, b2)


def kernel(**inputs) -> np.ndarray:
    flat, k, in_maps, prim, guest, unplaced, wb = _prepare(inputs)
    W1, b1, W2, b2 = wb
    if not _have_axon_devices():
        # no trn2 cores visible — compute on host so we still return the
        # right answer
        out = np.zeros((_T, _E), np.float64)
        for c in range(_NCORES):
            out[prim[c]] += _host_expert(flat[prim[c]], W1[c], b1[c], W2[c], b2[c]) / k
            if guest[c] is not None:
                ge, gidx = guest[c]
                out[gidx] += _host_expert(flat[gidx], W1[ge], b1[ge], W2[ge], b2[ge]) / k
        for e, idx in unplaced:
            out[idx] += _host_expert(flat[idx], W1[e], b1[e], W2[e], b2[e]) / k
        return out.astype(np.float32).reshape(_B, _SEQ, _E)
    # tokens that didn't fit the device layout (none for the benchmark
    # routing) are recomputed exactly on host
    extra = [(idx, _host_expert(flat[idx], W1[e], b1[e], W2[e], b2[e]) / k)
             for e, idx in unplaced]

    key = (float(1.0 / k),)
    if key not in _nc_cache:
        nc = _build_nc(1.0 / k)
        _nc_cache[key] = _make_runner(nc, _NCORES)
    run = _nc_cache[key]
    results = run(in_maps)

    out = np.zeros((_T, _E), np.float32)
    for c in range(_NCORES):
        y = results[c]["y"]
        out[prim[c]] += y[: len(prim[c])].astype(np.float32)
        if guest[c] is not None:
            ge, gidx = guest[c]
            out[gidx] += y[_CAPP : _CAPP + len(gidx)].astype(np.float32)
    for idx, yv in extra:
        out[idx] += yv.astype(np.float32)
    return out.reshape(_B, _SEQ, _E)
